# revision 21
# baseline (speedup 1.0000x reference)
"""MultiHeadSelfAttention2D Trainium2 kernel.

Full inputs -> shard batch (B=8) across 8 NeuronCores (1 image per core) ->
bass/Tile flash-attention-style kernel per core -> gather.

Per-core dataflow (feature-major, C=128 partitions, N=4096 tokens):
  Qf = (s*Wq) @ x + s*bq     (128 x N)   s = 1/sqrt(head_dim), folded on host
  Kf = Wk @ x + bk           (128 x N)
  Vaug = token-major V per (key-block, head): [V_h(32) | 1 | 0*31] so the
         PV matmul also accumulates the softmax denominator Z (M=64 col-
         tiled pairs -> two heads per PSUM bank, zeros keep dead rows clean)
  per (qblk, kblk, head):  S_T = Kf_h[kblk].T @ Qf_h[qblk]  (keys x queries,
                                 4 heads row-tiled concurrent on the PE)
                           P_T = exp(S_T)   (ACT engine, 2-bank-wide ops
                                 over a 4-bank PSUM ring = the bottleneck)
                           O_h += Vaug_h . P_T  (one PSUM bank per head,
                                 M=33, accumulated over kblks)
  1/Z: Z row at PSUM partition 32 of each head bank; copy to SBUF, bounce
       through DRAM to compact (128x16), exact reciprocal, bounce back,
       DMA per-head with a partition-broadcast AP -> rh (32 x 512).
  of_h = O_h * rh (DVE);  out = sum_h wo_h.T @ of_h + (bo + Wo @ bv)

Attention matmuls (QK, PV) run in bf16 (full-rate streaming + fast weight
load); projections run in float32r (TF32-like). fp32r requires every producer
to emit fp32r-typed outputs, hence the dtype plumbing on inputs/evictions.
"""

import numpy as np

EMBED = 128
HEADS = 4
HD = 32
P = 128

_CACHE = {}

# Schraudolph bf16 exp: i16 = round(S * EXPA + EXPB) bitcast to bf16
# (~3.3% max per-element error; end-to-end ~2e-3 after softmax).  Odd
# (kblk,head)-pair groups go to the Vector engine this way, halving the
# ScalarE exp bottleneck.
_EXP_C = 0.0430
EXPA = float(128.0 * 1.4426950408889634)
EXPB = float(128.0 * (127.0 - _EXP_C))
DVE_EXP = True

# This container's walrus build only accepts one sync-wait per Drain
# instruction; Tile's tail drain carries one wait per live proc. Split the
# extra waits onto standalone EventSemaphore instructions (same engine,
# earlier in program order - semantically identical).
_DRAIN_MAX_WAITS = 1


def _split_multiwait_drains(nc, mybir, bass_rust):
    """Move excess sync-waits onto standalone same-engine EventSemaphore
    instructions (the dispatching sequencer executes them in program order,
    so this is semantically identical)."""
    for fn in nc.m.functions:
        for bb in fn.blocks:
            new = []
            changed = False
            for inst in bb.instructions:
                si = inst.sync_info
                if (si is not None
                        and len(si.on_wait) > _DRAIN_MAX_WAITS):
                    changed = True
                    waits = list(si.on_wait)
                    for j, w in enumerate(waits[_DRAIN_MAX_WAITS:]):
                        es = mybir.InstEventSemaphore(
                            name=f"{inst.name}-wsplit{j}", ins=[], outs=[])
                        es.engine = inst.engine
                        es.sync_info = bass_rust.SyncInfo(
                            on_wait=[w], on_update=[])
                        nc.register_instruction(es)
                        new.append(es)
                    inst.sync_info = bass_rust.SyncInfo(
                        on_wait=waits[:_DRAIN_MAX_WAITS],
                        on_update=list(si.on_update))
                new.append(inst)
            if changed:
                bb.instructions = new


def _build_nc(n_tokens):
    import bass_rust
    import concourse.bass as bass
    import concourse.tile as tile
    import concourse.mybir as mybir

    f32 = mybir.dt.float32
    f32r = mybir.dt.float32r
    bf16 = mybir.dt.bfloat16
    i16 = mybir.dt.int16
    AF = mybir.ActivationFunctionType
    ADD = mybir.AluOpType.add
    MUL = mybir.AluOpType.mult
    N = n_tokens
    NQB = N // 512          # query blocks of 512
    NKB = N // 128          # key blocks of 128

    nc = bass.Bass("TRN2", target_bir_lowering=False, debug=False)

    x_d = nc.dram_tensor("x", [P, N], f32r, kind="ExternalInput").ap()
    wq_d = nc.dram_tensor("wq_t", [P, P], f32r, kind="ExternalInput").ap()
    wk_d = nc.dram_tensor("wk_t", [P, P], f32r, kind="ExternalInput").ap()
    wv_d = nc.dram_tensor("wv_t", [P, P], f32r, kind="ExternalInput").ap()
    wo_d = [nc.dram_tensor(f"wo_{h}", [32, P], f32r,
                           kind="ExternalInput").ap() for h in range(HEADS)]
    bq_d = nc.dram_tensor("bq", [P, 1], f32, kind="ExternalInput").ap()
    bk_d = nc.dram_tensor("bk", [P, 1], f32, kind="ExternalInput").ap()
    bo_d = nc.dram_tensor("bo", [P, 1], f32, kind="ExternalInput").ap()
    vones_d = nc.dram_tensor("vones", [P, P], bf16, kind="ExternalInput").ap()
    out_d = nc.dram_tensor("out", [P, N], f32, kind="ExternalOutput").ap()
    scr_d = [nc.dram_tensor(f"zscr{i}", [1, 2048], f32, kind="Internal").ap()
             for i in range(NQB)]
    scr2_d = [nc.dram_tensor(f"rscr{i}", [1, 2048], f32, kind="Internal").ap()
              for i in range(NQB)]

    with tile.TileContext(nc) as tc:
        _frees = []

        def ptile(shape, name, dt=None):
            t, f = tc.tile(shape, dt or f32, name=name)
            _frees.append(f)
            return t

        # ---- persistent SBUF tensors ----
        wq = ptile([P, P], "wq", f32r)
        wk = ptile([P, P], "wk", f32r)
        wv = ptile([P, P], "wv", f32r)
        wo = [ptile([32, P], f"wo{h}", f32r) for h in range(HEADS)]
        bq = ptile([P, 1], "bq_t")
        bk = ptile([P, 1], "bk_t")
        bo = ptile([P, 1], "bo_t")
        qf = ptile([P, N], "qf", bf16)
        kf = ptile([P, N], "kf", bf16)
        vaug = ptile([P, NKB * 132], "vaug", bf16)
        of = [ptile([32, N], f"of{h}", f32r) for h in range(HEADS)]

        nc.sync.dma_start(wq[:], wq_d)
        nc.sync.dma_start(wk[:], wk_d)
        nc.sync.dma_start(wv[:], wv_d)
        for h in range(HEADS):
            nc.sync.dma_start(wo[h][:], wo_d[h])
        nc.sync.dma_start(bq[:], bq_d)
        nc.sync.dma_start(bk[:], bk_d)
        nc.sync.dma_start(bo[:], bo_d)

        xt = []
        _xfrees = []
        for i in range(N // 512):
            t, xf = tc.tile([P, 512], f32r, name=f"x{i}")
            _xfrees.append(xf)
            nc.sync.dma_start(t[:], x_d[:, i * 512:(i + 1) * 512])
            xt.append(t)


        # vaug: per kblk a 132-wide block of 4x [Vh(32) | 1]
        ones_ap = vaug[:].rearrange(
            "p (j c) -> p j c", c=33)[:, :, 32]
        nc.sync.dma_start(ones_ap, vones_d[:, 0:NKB * HEADS])

        # ---- phase 1: projections ----
        with tc.tile_pool(name="pproj", bufs=2, space="PSUM") as pproj:
            for i in range(NQB):
                ps = pproj.tile([P, 512], f32, tag="ps")
                nc.tensor.matmul(ps[:], wq[:], xt[i][:], start=True, stop=True)
                nc.vector.tensor_scalar(
                    qf[:, i * 512:(i + 1) * 512], ps[:], bq[:], None, ADD)
                ps2 = pproj.tile([P, 512], f32, tag="ps")
                nc.tensor.matmul(ps2[:], wk[:], xt[i][:], start=True, stop=True)
                nc.vector.tensor_scalar(
                    kf[:, i * 512:(i + 1) * 512], ps2[:], bk[:], None, ADD)
            for k in range(NKB):
                vp = pproj.tile([P, 128], f32, tag="vp")
                xsl = xt[k // 4][:, (k % 4) * 128:(k % 4 + 1) * 128]
                nc.tensor.matmul(vp[:], xsl, wv[:], start=True, stop=True)
                dst = vaug[:, k * 132:(k + 1) * 132].rearrange(
                    "p (h c) -> p h c", h=HEADS)[:, :, 0:32]
                src = vp[:].rearrange("p (h c) -> p h c", h=HEADS)
                nc.vector.tensor_copy(dst, src)
        for xf in reversed(_xfrees):
            xf()

        # ---- phase 2: attention ----
        with tc.tile_pool(name="sring", bufs=1, space="PSUM") as sp, \
             tc.tile_pool(name="opool", bufs=1, space="PSUM") as opl, \
             tc.tile_pool(name="ptpool", bufs=9) as ptp, \
             tc.tile_pool(name="nrm1", bufs=1) as nrm1, \
             tc.tile_pool(name="nrm2", bufs=2) as nrm2:
            sring = sp.tile([P, 2048], f32)
            for q in range(NQB):
                ot = opl.tile([P, 2048], f32, tag="ot")
                pending = []
                units = [(k, h) for k in range(NKB) for h in range(HEADS)]
                for idx, (k, h) in enumerate(units):
                    bank = idx % 4
                    lhsT = kf[32 * h:32 * (h + 1), k * 128:(k + 1) * 128]
                    rhs = qf[32 * h:32 * (h + 1), q * 512:(q + 1) * 512]
                    nc.tensor.matmul(
                        sring[:, bank * 512:(bank + 1) * 512], lhsT, rhs,
                        start=True, stop=True, tile_position=(32 * h, 0))
                    pending.append((bank, k, h))
                    if len(pending) == 2 or idx == len(units) - 1:
                        b0 = pending[0][0]
                        w = len(pending) * 512
                        ptt = ptp.tile([P, 1024], bf16, tag="pt")
                        src_ap = sring[:, b0 * 512:b0 * 512 + w]
                        if DVE_EXP and (idx // 2) % 2 == 1:
                            nc.vector.tensor_scalar(
                                ptt[:, 0:w].bitcast(i16), src_ap,
                                EXPA, EXPB, MUL, ADD)
                        else:
                            nc.scalar.activation(ptt[:, 0:w], src_ap, AF.Exp)
                        for j, (_, kk, hh) in enumerate(pending):
                            rhs2 = ptt[:, j * 512:(j + 1) * 512]
                            lhs2 = vaug[:, kk * 132 + 33 * hh:
                                        kk * 132 + 33 * hh + 33]
                            out2 = ot[0:33, hh * 512:hh * 512 + 512]
                            nc.tensor.matmul(
                                out2, lhs2, rhs2,
                                start=(kk == 0), stop=(kk == NKB - 1),
                                skip_group_check=True)
                        pending = []

                # ---- normalize ----
                # Z row: psum partition 32 of each head bank ->
                # [Z_h0 | Z_h1 | Z_h2 | Z_h3] (512 each). Bounce through
                # DRAM to reshape compact, reciprocal on 128 lanes,
                # bounce back, broadcast per head, multiply.
                zrow = nrm1.tile([1, 2048], f32, tag="zrow")
                nc.vector.tensor_copy(zrow[:], ot[32:33, :])
                scr = scr_d[q]
                nc.sync.dma_start(scr, zrow[:])
                cmp_ = nrm1.tile([P, 16], f32, tag="cmp")
                nc.sync.dma_start(
                    cmp_[:], scr.rearrange("o (p f) -> (o p) f", p=P))
                cmpr = nrm1.tile([P, 16], f32, tag="cmpr")
                nc.vector.reciprocal(cmpr[:], cmp_[:])
                scr2 = scr2_d[q]
                nc.sync.dma_start(
                    scr2.rearrange("o (p f) -> (o p) f", p=P), cmpr[:])
                qs = slice(q * 512, (q + 1) * 512)
                for h in range(HEADS):
                    rh = nrm2.tile([32, 512], f32, tag=f"rh{h}")
                    nc.sync.dma_start(
                        rh[:], scr2[0:1, 512 * h:512 * (h + 1)]
                        .partition_broadcast(32))
                    nc.vector.tensor_mul(
                        of[h][:, qs], ot[0:32, h * 512:(h + 1) * 512], rh[:])

        # ---- phase 3: output projection ----
        with tc.tile_pool(name="pout", bufs=2, space="PSUM") as pop, \
             tc.tile_pool(name="osb", bufs=2) as osbp:
            for i in range(NQB):
                po = pop.tile([P, 512], f32, tag="po")
                for h in range(HEADS):
                    nc.tensor.matmul(po[:], wo[h][:],
                                     of[h][:, i * 512:(i + 1) * 512],
                                     start=(h == 0), stop=(h == HEADS - 1))
                ob = osbp.tile([P, 512], f32, tag="ob")
                nc.vector.tensor_scalar(ob[:], po[:], bo[:], None, ADD)
                nc.sync.dma_start(out_d[:, i * 512:(i + 1) * 512], ob[:])

        for f in reversed(_frees):
            f()

    _split_multiwait_drains(nc, mybir, bass_rust)
    return nc


def prep_weights(Wq, bq, Wk, bk, Wv, bv, Wo, bo):
    """Host-side weight preprocessing (all fp32 numpy)."""
    s = np.float32(1.0 / np.sqrt(HD))
    wq_t = np.ascontiguousarray((s * Wq).T)
    bq_s = np.ascontiguousarray((s * bq).reshape(P, 1))
    wk_t = np.ascontiguousarray(Wk.T)
    bk_c = np.ascontiguousarray(bk.reshape(P, 1))
    wv_t = np.ascontiguousarray(Wv.T)
    bo_f = np.ascontiguousarray((bo + Wo @ bv).reshape(P, 1)).astype(np.float32)
    import ml_dtypes
    d = dict(wq_t=wq_t, bq=bq_s, wk_t=wk_t, bk=bk_c, wv_t=wv_t, bo=bo_f,
             vones=np.ones((P, P), ml_dtypes.bfloat16))
    for h in range(HEADS):
        d[f"wo_{h}"] = np.ascontiguousarray(Wo[:, 32 * h:32 * (h + 1)].T)
    return d


LAST_RESULTS = None


def kernel(x, Wq, bq, Wk, bk, Wv, bv, Wo, bo):
    global LAST_RESULTS
    import os
    from concourse.bass_utils import run_bass_kernel_spmd

    x = np.asarray(x, np.float32)
    B, C, H, W = x.shape
    N = H * W
    key = ("nc", N)
    if key not in _CACHE:
        _CACHE[key] = _build_nc(N)
    nc = _CACHE[key]

    wmap = prep_weights(np.asarray(Wq, np.float32), np.asarray(bq, np.float32),
                        np.asarray(Wk, np.float32), np.asarray(bk, np.float32),
                        np.asarray(Wv, np.float32), np.asarray(bv, np.float32),
                        np.asarray(Wo, np.float32), np.asarray(bo, np.float32))

    in_maps = []
    for b in range(B):
        m = dict(wmap)
        m["x"] = np.ascontiguousarray(x[b].reshape(C, N))
        in_maps.append(m)

    tmpdir = os.environ.get("KERNEL_TMPDIR") or None
    res = run_bass_kernel_spmd(nc, in_maps, core_ids=list(range(B)),
                               tmpdir=tmpdir)
    LAST_RESULTS = res
    out = np.stack([res.results[b]["out"] for b in range(B)], axis=0)
    return out.reshape(B, C, H, W).astype(np.float32)



# revision 25
# speedup vs baseline: 1.3643x; 1.3643x over previous
"""MultiHeadSelfAttention2D Trainium2 kernel.

Full inputs -> shard batch (B=8) across 8 NeuronCores (1 image per core) ->
bass/Tile flash-attention-style kernel per core -> gather.

Per-core dataflow (feature-major, C=128 partitions, N=4096 tokens):
  Qf = (s*Wq) @ x + s*bq     (128 x N)   s = 1/sqrt(head_dim), folded on host
  Kf = Wk @ x + bk           (128 x N)
  Vaug = token-major V per (key-block, head): [V_h(32) | 1 | 0*31] so the
         PV matmul also accumulates the softmax denominator Z (M=64 col-
         tiled pairs -> two heads per PSUM bank, zeros keep dead rows clean)
  per (qblk, kblk, head):  S_T = Kf_h[kblk].T @ Qf_h[qblk]  (keys x queries,
                                 4 heads row-tiled concurrent on the PE)
                           P_T = exp(S_T)   (ACT engine, 2-bank-wide ops
                                 over a 4-bank PSUM ring = the bottleneck)
                           O_h += Vaug_h . P_T  (one PSUM bank per head,
                                 M=33, accumulated over kblks)
  1/Z: Z row at PSUM partition 32 of each head bank; copy to SBUF, bounce
       through DRAM to compact (128x16), exact reciprocal, bounce back,
       DMA per-head with a partition-broadcast AP -> rh (32 x 512).
  of_h = O_h * rh (DVE);  out = sum_h wo_h.T @ of_h + (bo + Wo @ bv)

Attention matmuls (QK, PV) run in bf16 (full-rate streaming + fast weight
load); projections run in float32r (TF32-like). fp32r requires every producer
to emit fp32r-typed outputs, hence the dtype plumbing on inputs/evictions.
"""

import numpy as np

EMBED = 128
HEADS = 4
HD = 32
P = 128

_CACHE = {}

# Schraudolph bf16 exp: i16 = round(S * EXPA + EXPB) bitcast to bf16
# (~3.3% max per-element error; end-to-end ~2e-3 after softmax).  Odd
# (kblk,head)-pair groups go to the Vector engine this way, halving the
# ScalarE exp bottleneck.
_EXP_C = 0.0430
EXPA = float(128.0 * 1.4426950408889634)
EXPB = float(128.0 * (127.0 - _EXP_C))
DVE_EXP = True

# This container's walrus build only accepts one sync-wait per Drain
# instruction; Tile's tail drain carries one wait per live proc. Split the
# extra waits onto standalone EventSemaphore instructions (same engine,
# earlier in program order - semantically identical).
_DRAIN_MAX_WAITS = 1


def _split_multiwait_drains(nc, mybir, bass_rust):
    """Move excess sync-waits onto standalone same-engine EventSemaphore
    instructions (the dispatching sequencer executes them in program order,
    so this is semantically identical)."""
    for fn in nc.m.functions:
        for bb in fn.blocks:
            new = []
            changed = False
            for inst in bb.instructions:
                si = inst.sync_info
                if (si is not None
                        and len(si.on_wait) > _DRAIN_MAX_WAITS):
                    changed = True
                    waits = list(si.on_wait)
                    for j, w in enumerate(waits[_DRAIN_MAX_WAITS:]):
                        es = mybir.InstEventSemaphore(
                            name=f"{inst.name}-wsplit{j}", ins=[], outs=[])
                        es.engine = inst.engine
                        es.sync_info = bass_rust.SyncInfo(
                            on_wait=[w], on_update=[])
                        nc.register_instruction(es)
                        new.append(es)
                    inst.sync_info = bass_rust.SyncInfo(
                        on_wait=waits[:_DRAIN_MAX_WAITS],
                        on_update=list(si.on_update))
                new.append(inst)
            if changed:
                bb.instructions = new


def _build_nc(n_tokens):
    import bass_rust
    import concourse.bass as bass
    import concourse.tile as tile
    import concourse.mybir as mybir

    f32 = mybir.dt.float32
    f32r = mybir.dt.float32r
    bf16 = mybir.dt.bfloat16
    i16 = mybir.dt.int16
    AF = mybir.ActivationFunctionType
    ADD = mybir.AluOpType.add
    MUL = mybir.AluOpType.mult
    N = n_tokens
    NQB = N // 512          # query blocks of 512
    NKB = N // 128          # key blocks of 128

    nc = bass.Bass("TRN2", target_bir_lowering=False, debug=False)

    x_d = nc.dram_tensor("x", [P, N], f32r, kind="ExternalInput").ap()
    wq_d = nc.dram_tensor("wq_t", [P, P], f32r, kind="ExternalInput").ap()
    wk_d = nc.dram_tensor("wk_t", [P, P], f32r, kind="ExternalInput").ap()
    wv_d = nc.dram_tensor("wv_t", [P, P], f32r, kind="ExternalInput").ap()
    wo_d = [nc.dram_tensor(f"wo_{h}", [32, P], f32r,
                           kind="ExternalInput").ap() for h in range(HEADS)]
    bq_d = nc.dram_tensor("bq", [P, 1], f32, kind="ExternalInput").ap()
    bk_d = nc.dram_tensor("bk", [P, 1], f32, kind="ExternalInput").ap()
    bo_d = nc.dram_tensor("bo", [P, 1], f32, kind="ExternalInput").ap()
    vones_d = nc.dram_tensor("vones", [P, P], bf16, kind="ExternalInput").ap()
    out_d = nc.dram_tensor("out", [P, N], f32, kind="ExternalOutput").ap()
    scr_d = [nc.dram_tensor(f"zscr{i}", [1, 2048], f32, kind="Internal").ap()
             for i in range(NQB)]
    scr2_d = [nc.dram_tensor(f"rscr{i}", [1, 2048], f32, kind="Internal").ap()
              for i in range(NQB)]

    with tile.TileContext(nc) as tc:
        _frees = []

        def ptile(shape, name, dt=None):
            t, f = tc.tile(shape, dt or f32, name=name)
            _frees.append(f)
            return t

        # ---- persistent SBUF tensors ----
        wq = ptile([P, P], "wq", f32r)
        wk = ptile([P, P], "wk", f32r)
        wv = ptile([P, P], "wv", f32r)
        wo = [ptile([32, P], f"wo{h}", f32r) for h in range(HEADS)]
        bq = ptile([P, 1], "bq_t")
        bk = ptile([P, 1], "bk_t")
        bo = ptile([P, 1], "bo_t")
        qf = ptile([P, N], "qf", bf16)
        kf = ptile([P, N], "kf", bf16)
        vaug = ptile([P, NKB * 132], "vaug", bf16)
        of = [ptile([32, N], f"of{h}", f32r) for h in range(HEADS)]

        nc.sync.dma_start(wq[:], wq_d)
        nc.sync.dma_start(wk[:], wk_d)
        nc.sync.dma_start(wv[:], wv_d)
        for h in range(HEADS):
            nc.sync.dma_start(wo[h][:], wo_d[h])
        nc.sync.dma_start(bq[:], bq_d)
        nc.sync.dma_start(bk[:], bk_d)
        nc.sync.dma_start(bo[:], bo_d)

        xt = []
        _xfrees = []
        for i in range(N // 512):
            t, xf = tc.tile([P, 512], f32r, name=f"x{i}")
            _xfrees.append(xf)
            nc.sync.dma_start(t[:], x_d[:, i * 512:(i + 1) * 512])
            xt.append(t)


        # vaug: per kblk a 132-wide block of 4x [Vh(32) | 1]
        ones_ap = vaug[:].rearrange(
            "p (j c) -> p j c", c=33)[:, :, 32]
        nc.sync.dma_start(ones_ap, vones_d[:, 0:NKB * HEADS])

        # ---- phase 1: projections ----
        with tc.tile_pool(name="pproj", bufs=2, space="PSUM") as pproj:
            for i in range(NQB):
                ps = pproj.tile([P, 512], f32, tag="ps")
                nc.tensor.matmul(ps[:], wq[:], xt[i][:], start=True, stop=True)
                nc.vector.tensor_scalar(
                    qf[:, i * 512:(i + 1) * 512], ps[:], bq[:], None, ADD)
                ps2 = pproj.tile([P, 512], f32, tag="ps")
                nc.tensor.matmul(ps2[:], wk[:], xt[i][:], start=True, stop=True)
                nc.vector.tensor_scalar(
                    kf[:, i * 512:(i + 1) * 512], ps2[:], bk[:], None, ADD)
            for k in range(NKB):
                vp = pproj.tile([P, 128], f32, tag="vp")
                xsl = xt[k // 4][:, (k % 4) * 128:(k % 4 + 1) * 128]
                nc.tensor.matmul(vp[:], xsl, wv[:], start=True, stop=True)
                dst = vaug[:, k * 132:(k + 1) * 132].rearrange(
                    "p (h c) -> p h c", h=HEADS)[:, :, 0:32]
                src = vp[:].rearrange("p (h c) -> p h c", h=HEADS)
                nc.vector.tensor_copy(dst, src)
        for xf in reversed(_xfrees):
            xf()

        # ---- phase 2: attention ----
        with tc.tile_pool(name="sring", bufs=1, space="PSUM") as sp, \
             tc.tile_pool(name="opool", bufs=1, space="PSUM") as opl, \
             tc.tile_pool(name="ptpool", bufs=9) as ptp, \
             tc.tile_pool(name="nrm1", bufs=1) as nrm1, \
             tc.tile_pool(name="nrm2", bufs=2) as nrm2:
            sring = sp.tile([P, 2048], f32)
            PV_LAG = 2      # groups the PV matmuls trail the QK/exp stream

            for q in range(NQB):
                ot = opl.tile([P, 2048], f32, tag="ot")
                # groups of 2 units: (k, h-pair); each group = one 2-bank
                # ring slot, one exp op (engines alternate), two PV matmuls
                # emitted PV_LAG groups later so the PE stream runs ahead
                # of the exp engines and the two exp engines overlap.
                groups = [(k, hf) for k in range(NKB) for hf in (0, 1)]
                pts = {}

                def emit_pv(gi):
                    k, hf = groups[gi]
                    ptt = pts.pop(gi)
                    for j in (0, 1):
                        h = 2 * hf + j
                        rhs2 = ptt[:, j * 512:(j + 1) * 512]
                        lhs2 = vaug[:, k * 132 + 33 * h:
                                    k * 132 + 33 * h + 33]
                        out2 = ot[0:33, h * 512:h * 512 + 512]
                        nc.tensor.matmul(
                            out2, lhs2, rhs2,
                            start=(k == 0), stop=(k == NKB - 1),
                            skip_group_check=True)

                for gi, (k, hf) in enumerate(groups):
                    b = (2 * gi) % 4
                    for j in (0, 1):
                        h = 2 * hf + j
                        lhsT = kf[32 * h:32 * (h + 1),
                                  k * 128:(k + 1) * 128]
                        rhs = qf[32 * h:32 * (h + 1),
                                 q * 512:(q + 1) * 512]
                        nc.tensor.matmul(
                            sring[:, (b + j) * 512:(b + j + 1) * 512],
                            lhsT, rhs,
                            start=True, stop=True,
                            tile_position=(32 * h, 0))
                    ptt = ptp.tile([P, 1024], bf16, tag="pt")
                    pts[gi] = ptt
                    src_ap = sring[:, b * 512:(b + 2) * 512]
                    if DVE_EXP and gi % 2 == 1:
                        nc.vector.tensor_scalar(
                            ptt[:].bitcast(i16), src_ap,
                            EXPA, EXPB, MUL, ADD)
                    else:
                        nc.scalar.activation(ptt[:], src_ap, AF.Exp)
                    if gi >= PV_LAG:
                        emit_pv(gi - PV_LAG)
                for gi in range(len(groups) - PV_LAG, len(groups)):
                    emit_pv(gi)

                # ---- normalize ----
                # Z row: psum partition 32 of each head bank ->
                # [Z_h0 | Z_h1 | Z_h2 | Z_h3] (512 each). Bounce through
                # DRAM to reshape compact, reciprocal on 128 lanes,
                # bounce back, broadcast per head, multiply.
                zrow = nrm1.tile([1, 2048], f32, tag="zrow")
                nc.vector.tensor_copy(zrow[:], ot[32:33, :])
                scr = scr_d[q]
                nc.sync.dma_start(scr, zrow[:])
                cmp_ = nrm1.tile([P, 16], f32, tag="cmp")
                nc.sync.dma_start(
                    cmp_[:], scr.rearrange("o (p f) -> (o p) f", p=P))
                cmpr = nrm1.tile([P, 16], f32, tag="cmpr")
                nc.vector.reciprocal(cmpr[:], cmp_[:])
                scr2 = scr2_d[q]
                nc.sync.dma_start(
                    scr2.rearrange("o (p f) -> (o p) f", p=P), cmpr[:])
                qs = slice(q * 512, (q + 1) * 512)
                for h in range(HEADS):
                    rh = nrm2.tile([32, 512], f32, tag=f"rh{h}")
                    nc.sync.dma_start(
                        rh[:], scr2[0:1, 512 * h:512 * (h + 1)]
                        .partition_broadcast(32))
                    nc.vector.tensor_mul(
                        of[h][:, qs], ot[0:32, h * 512:(h + 1) * 512], rh[:])

        # ---- phase 3: output projection ----
        with tc.tile_pool(name="pout", bufs=2, space="PSUM") as pop, \
             tc.tile_pool(name="osb", bufs=2) as osbp:
            for i in range(NQB):
                po = pop.tile([P, 512], f32, tag="po")
                for h in range(HEADS):
                    nc.tensor.matmul(po[:], wo[h][:],
                                     of[h][:, i * 512:(i + 1) * 512],
                                     start=(h == 0), stop=(h == HEADS - 1))
                ob = osbp.tile([P, 512], f32, tag="ob")
                nc.vector.tensor_scalar(ob[:], po[:], bo[:], None, ADD)
                nc.sync.dma_start(out_d[:, i * 512:(i + 1) * 512], ob[:])

        for f in reversed(_frees):
            f()

    _split_multiwait_drains(nc, mybir, bass_rust)
    return nc


def prep_weights(Wq, bq, Wk, bk, Wv, bv, Wo, bo):
    """Host-side weight preprocessing (all fp32 numpy)."""
    s = np.float32(1.0 / np.sqrt(HD))
    wq_t = np.ascontiguousarray((s * Wq).T)
    bq_s = np.ascontiguousarray((s * bq).reshape(P, 1))
    wk_t = np.ascontiguousarray(Wk.T)
    bk_c = np.ascontiguousarray(bk.reshape(P, 1))
    wv_t = np.ascontiguousarray(Wv.T)
    bo_f = np.ascontiguousarray((bo + Wo @ bv).reshape(P, 1)).astype(np.float32)
    import ml_dtypes
    d = dict(wq_t=wq_t, bq=bq_s, wk_t=wk_t, bk=bk_c, wv_t=wv_t, bo=bo_f,
             vones=np.ones((P, P), ml_dtypes.bfloat16))
    for h in range(HEADS):
        d[f"wo_{h}"] = np.ascontiguousarray(Wo[:, 32 * h:32 * (h + 1)].T)
    return d


LAST_RESULTS = None


def kernel(x, Wq, bq, Wk, bk, Wv, bv, Wo, bo):
    global LAST_RESULTS
    import os
    from concourse.bass_utils import run_bass_kernel_spmd

    x = np.asarray(x, np.float32)
    B, C, H, W = x.shape
    N = H * W
    key = ("nc", N)
    if key not in _CACHE:
        _CACHE[key] = _build_nc(N)
    nc = _CACHE[key]

    wmap = prep_weights(np.asarray(Wq, np.float32), np.asarray(bq, np.float32),
                        np.asarray(Wk, np.float32), np.asarray(bk, np.float32),
                        np.asarray(Wv, np.float32), np.asarray(bv, np.float32),
                        np.asarray(Wo, np.float32), np.asarray(bo, np.float32))

    in_maps = []
    for b in range(B):
        m = dict(wmap)
        m["x"] = np.ascontiguousarray(x[b].reshape(C, N))
        in_maps.append(m)

    tmpdir = os.environ.get("KERNEL_TMPDIR") or None
    res = run_bass_kernel_spmd(nc, in_maps, core_ids=list(range(B)),
                               tmpdir=tmpdir)
    LAST_RESULTS = res
    out = np.stack([res.results[b]["out"] for b in range(B)], axis=0)
    return out.reshape(B, C, H, W).astype(np.float32)



# revision 26
# speedup vs baseline: 2.3916x; 1.7529x over previous
"""MultiHeadSelfAttention2D Trainium2 kernel.

Full inputs -> shard batch (B=8) across 8 NeuronCores (1 image per core) ->
bass/Tile flash-attention-style kernel per core -> gather.

Per-core dataflow (feature-major, C=128 partitions, N=4096 tokens):
  Qf = (s*Wq) @ x + s*bq     (128 x N)   s = 1/sqrt(head_dim), folded on host
  Kf = Wk @ x + bk           (128 x N)
  Vaug = token-major V per (key-block, head): [V_h(32) | 1 | 0*31] so the
         PV matmul also accumulates the softmax denominator Z (M=64 col-
         tiled pairs -> two heads per PSUM bank, zeros keep dead rows clean)
  per (qblk, kblk, head):  S_T = Kf_h[kblk].T @ Qf_h[qblk]  (keys x queries,
                                 4 heads row-tiled concurrent on the PE)
                           P_T = exp(S_T)   (ACT engine, 2-bank-wide ops
                                 over a 4-bank PSUM ring = the bottleneck)
                           O_h += Vaug_h . P_T  (one PSUM bank per head,
                                 M=33, accumulated over kblks)
  1/Z: Z row at PSUM partition 32 of each head bank; copy to SBUF, bounce
       through DRAM to compact (128x16), exact reciprocal, bounce back,
       DMA per-head with a partition-broadcast AP -> rh (32 x 512).
  of_h = O_h * rh (DVE);  out = sum_h wo_h.T @ of_h + (bo + Wo @ bv)

Attention matmuls (QK, PV) run in bf16 (full-rate streaming + fast weight
load); projections run in float32r (TF32-like). fp32r requires every producer
to emit fp32r-typed outputs, hence the dtype plumbing on inputs/evictions.
"""

import numpy as np

EMBED = 128
HEADS = 4
HD = 32
P = 128

_CACHE = {}

# Schraudolph bf16 exp: i16 = round(S * EXPA + EXPB) bitcast to bf16
# (~3.3% max per-element error; end-to-end ~2e-3 after softmax).  Odd
# (kblk,head)-pair groups go to the Vector engine this way, halving the
# ScalarE exp bottleneck.
_EXP_C = 0.0430
EXPA = float(128.0 * 1.4426950408889634)
EXPB = float(128.0 * (127.0 - _EXP_C))
DVE_EXP = True

# This container's walrus build only accepts one sync-wait per Drain
# instruction; Tile's tail drain carries one wait per live proc. Split the
# extra waits onto standalone EventSemaphore instructions (same engine,
# earlier in program order - semantically identical).
_DRAIN_MAX_WAITS = 1


def _split_multiwait_drains(nc, mybir, bass_rust):
    """Move excess sync-waits onto standalone same-engine EventSemaphore
    instructions (the dispatching sequencer executes them in program order,
    so this is semantically identical)."""
    for fn in nc.m.functions:
        for bb in fn.blocks:
            new = []
            changed = False
            for inst in bb.instructions:
                si = inst.sync_info
                if (si is not None
                        and len(si.on_wait) > _DRAIN_MAX_WAITS):
                    changed = True
                    waits = list(si.on_wait)
                    for j, w in enumerate(waits[_DRAIN_MAX_WAITS:]):
                        es = mybir.InstEventSemaphore(
                            name=f"{inst.name}-wsplit{j}", ins=[], outs=[])
                        es.engine = inst.engine
                        es.sync_info = bass_rust.SyncInfo(
                            on_wait=[w], on_update=[])
                        nc.register_instruction(es)
                        new.append(es)
                    inst.sync_info = bass_rust.SyncInfo(
                        on_wait=waits[:_DRAIN_MAX_WAITS],
                        on_update=list(si.on_update))
                new.append(inst)
            if changed:
                bb.instructions = new


def _build_nc(n_tokens):
    import bass_rust
    import concourse.bass as bass
    import concourse.tile as tile
    import concourse.mybir as mybir

    f32 = mybir.dt.float32
    f32r = mybir.dt.float32r
    bf16 = mybir.dt.bfloat16
    i16 = mybir.dt.int16
    AF = mybir.ActivationFunctionType
    ADD = mybir.AluOpType.add
    MUL = mybir.AluOpType.mult
    N = n_tokens
    NQB = N // 512          # query blocks of 512
    NKB = N // 128          # key blocks of 128

    nc = bass.Bass("TRN2", target_bir_lowering=False, debug=False)

    x_d = nc.dram_tensor("x", [P, N], f32r, kind="ExternalInput").ap()
    wq_d = nc.dram_tensor("wq_t", [P, P], f32r, kind="ExternalInput").ap()
    wk_d = nc.dram_tensor("wk_t", [P, P], f32r, kind="ExternalInput").ap()
    wv_d = nc.dram_tensor("wv_t", [P, P], f32r, kind="ExternalInput").ap()
    wo_d = [nc.dram_tensor(f"wo_{h}", [32, P], f32r,
                           kind="ExternalInput").ap() for h in range(HEADS)]
    bq_d = nc.dram_tensor("bq", [P, 1], f32, kind="ExternalInput").ap()
    bk_d = nc.dram_tensor("bk", [P, 1], f32, kind="ExternalInput").ap()
    bo_d = nc.dram_tensor("bo", [P, 1], f32, kind="ExternalInput").ap()
    vones_d = nc.dram_tensor("vones", [P, P], bf16, kind="ExternalInput").ap()
    out_d = nc.dram_tensor("out", [P, N], f32, kind="ExternalOutput").ap()
    scr_d = [nc.dram_tensor(f"zscr{i}", [1, 2048], f32, kind="Internal").ap()
             for i in range(NQB)]
    scr2_d = [nc.dram_tensor(f"rscr{i}", [1, 2048], f32, kind="Internal").ap()
              for i in range(NQB)]

    with tile.TileContext(nc) as tc:
        _frees = []

        def ptile(shape, name, dt=None):
            t, f = tc.tile(shape, dt or f32, name=name)
            _frees.append(f)
            return t

        # ---- persistent SBUF tensors ----
        wq = ptile([P, P], "wq", f32r)
        wk = ptile([P, P], "wk", f32r)
        wv = ptile([P, P], "wv", f32r)
        wo = [ptile([32, P], f"wo{h}", f32r) for h in range(HEADS)]
        bq = ptile([P, 1], "bq_t")
        bk = ptile([P, 1], "bk_t")
        bo = ptile([P, 1], "bo_t")
        qf = ptile([P, N], "qf", bf16)
        kf = ptile([P, N], "kf", bf16)
        vaug = ptile([P, NKB * 132], "vaug", bf16)
        of = [ptile([32, N], f"of{h}", f32r) for h in range(HEADS)]

        nc.sync.dma_start(wq[:], wq_d)
        nc.sync.dma_start(wk[:], wk_d)
        nc.sync.dma_start(wv[:], wv_d)
        for h in range(HEADS):
            nc.sync.dma_start(wo[h][:], wo_d[h])
        nc.sync.dma_start(bq[:], bq_d)
        nc.sync.dma_start(bk[:], bk_d)
        nc.sync.dma_start(bo[:], bo_d)

        xt = []
        _xfrees = []
        for i in range(N // 512):
            t, xf = tc.tile([P, 512], f32r, name=f"x{i}")
            _xfrees.append(xf)
            nc.sync.dma_start(t[:], x_d[:, i * 512:(i + 1) * 512])
            xt.append(t)


        # vaug: per kblk a 132-wide block of 4x [Vh(32) | 1]
        ones_ap = vaug[:].rearrange(
            "p (j c) -> p j c", c=33)[:, :, 32]
        nc.sync.dma_start(ones_ap, vones_d[:, 0:NKB * HEADS])

        # ---- phase 1: projections ----
        with tc.tile_pool(name="pproj", bufs=2, space="PSUM") as pproj:
            for i in range(NQB):
                ps = pproj.tile([P, 512], f32, tag="ps")
                nc.tensor.matmul(ps[:], wq[:], xt[i][:], start=True, stop=True)
                nc.vector.tensor_scalar(
                    qf[:, i * 512:(i + 1) * 512], ps[:], bq[:], None, ADD)
                ps2 = pproj.tile([P, 512], f32, tag="ps")
                nc.tensor.matmul(ps2[:], wk[:], xt[i][:], start=True, stop=True)
                nc.vector.tensor_scalar(
                    kf[:, i * 512:(i + 1) * 512], ps2[:], bk[:], None, ADD)
            for k in range(NKB):
                vp = pproj.tile([P, 128], f32, tag="vp")
                xsl = xt[k // 4][:, (k % 4) * 128:(k % 4 + 1) * 128]
                nc.tensor.matmul(vp[:], xsl, wv[:], start=True, stop=True)
                dst = vaug[:, k * 132:(k + 1) * 132].rearrange(
                    "p (h c) -> p h c", h=HEADS)[:, :, 0:32]
                src = vp[:].rearrange("p (h c) -> p h c", h=HEADS)
                nc.vector.tensor_copy(dst, src)
        for xf in reversed(_xfrees):
            xf()

        # ---- phase 2: attention ----
        with tc.tile_pool(name="sring", bufs=1, space="PSUM") as sp, \
             tc.tile_pool(name="opool", bufs=1, space="PSUM") as opl, \
             tc.tile_pool(name="ptpool", bufs=9) as ptp, \
             tc.tile_pool(name="nrm1", bufs=1) as nrm1, \
             tc.tile_pool(name="nrm2", bufs=2) as nrm2:
            # two independent ring-slot tiles -> precise WAR tracking:
            # QK(g) only waits exp(g-2) (same slot), not every prior exp.
            slotA = sp.tile([P, 1024], f32)
            slotB = sp.tile([P, 1024], f32)
            slots = [slotA, slotB]
            PV_LAG = 2      # groups the PV matmuls trail the QK/exp stream

            for q in range(NQB):
                ot = opl.tile([P, 2048], f32, tag="ot")
                # groups of 2 units: (k, h-pair); each group = one 2-bank
                # ring slot, one exp op (engines alternate), two PV matmuls
                # emitted PV_LAG groups later so the PE stream runs ahead
                # of the exp engines and the two exp engines overlap.
                groups = [(k, hf) for k in range(NKB) for hf in (0, 1)]
                pts = {}

                def emit_pv(gi):
                    k, hf = groups[gi]
                    ptt = pts.pop(gi)
                    for j in (0, 1):
                        h = 2 * hf + j
                        rhs2 = ptt[:, j * 512:(j + 1) * 512]
                        lhs2 = vaug[:, k * 132 + 33 * h:
                                    k * 132 + 33 * h + 33]
                        out2 = ot[0:33, h * 512:h * 512 + 512]
                        nc.tensor.matmul(
                            out2, lhs2, rhs2,
                            start=(k == 0), stop=(k == NKB - 1),
                            skip_group_check=True)

                for gi, (k, hf) in enumerate(groups):
                    slot = slots[gi % 2]
                    for j in (0, 1):
                        h = 2 * hf + j
                        lhsT = kf[32 * h:32 * (h + 1),
                                  k * 128:(k + 1) * 128]
                        rhs = qf[32 * h:32 * (h + 1),
                                 q * 512:(q + 1) * 512]
                        nc.tensor.matmul(
                            slot[:, j * 512:(j + 1) * 512],
                            lhsT, rhs,
                            start=True, stop=True,
                            tile_position=(32 * h, 0))
                    ptt = ptp.tile([P, 1024], bf16, tag="pt")
                    pts[gi] = ptt
                    src_ap = slot[:]
                    if DVE_EXP and gi % 2 == 1:
                        nc.vector.tensor_scalar(
                            ptt[:].bitcast(i16), src_ap,
                            EXPA, EXPB, MUL, ADD)
                    else:
                        nc.scalar.activation(ptt[:], src_ap, AF.Exp)
                    if gi >= PV_LAG:
                        emit_pv(gi - PV_LAG)
                for gi in range(len(groups) - PV_LAG, len(groups)):
                    emit_pv(gi)

                # ---- normalize ----
                # Z row: psum partition 32 of each head bank ->
                # [Z_h0 | Z_h1 | Z_h2 | Z_h3] (512 each). Bounce through
                # DRAM to reshape compact, reciprocal on 128 lanes,
                # bounce back, broadcast per head, multiply.
                zrow = nrm1.tile([1, 2048], f32, tag="zrow")
                nc.vector.tensor_copy(zrow[:], ot[32:33, :])
                scr = scr_d[q]
                nc.sync.dma_start(scr, zrow[:])
                cmp_ = nrm1.tile([P, 16], f32, tag="cmp")
                nc.sync.dma_start(
                    cmp_[:], scr.rearrange("o (p f) -> (o p) f", p=P))
                cmpr = nrm1.tile([P, 16], f32, tag="cmpr")
                nc.vector.reciprocal(cmpr[:], cmp_[:])
                scr2 = scr2_d[q]
                nc.sync.dma_start(
                    scr2.rearrange("o (p f) -> (o p) f", p=P), cmpr[:])
                qs = slice(q * 512, (q + 1) * 512)
                for h in range(HEADS):
                    rh = nrm2.tile([32, 512], f32, tag=f"rh{h}")
                    nc.sync.dma_start(
                        rh[:], scr2[0:1, 512 * h:512 * (h + 1)]
                        .partition_broadcast(32))
                    nc.vector.tensor_mul(
                        of[h][:, qs], ot[0:32, h * 512:(h + 1) * 512], rh[:])

        # ---- phase 3: output projection ----
        with tc.tile_pool(name="pout", bufs=2, space="PSUM") as pop, \
             tc.tile_pool(name="osb", bufs=2) as osbp:
            for i in range(NQB):
                po = pop.tile([P, 512], f32, tag="po")
                for h in range(HEADS):
                    nc.tensor.matmul(po[:], wo[h][:],
                                     of[h][:, i * 512:(i + 1) * 512],
                                     start=(h == 0), stop=(h == HEADS - 1))
                ob = osbp.tile([P, 512], f32, tag="ob")
                nc.vector.tensor_scalar(ob[:], po[:], bo[:], None, ADD)
                nc.sync.dma_start(out_d[:, i * 512:(i + 1) * 512], ob[:])

        for f in reversed(_frees):
            f()

    _split_multiwait_drains(nc, mybir, bass_rust)
    return nc


def prep_weights(Wq, bq, Wk, bk, Wv, bv, Wo, bo):
    """Host-side weight preprocessing (all fp32 numpy)."""
    s = np.float32(1.0 / np.sqrt(HD))
    wq_t = np.ascontiguousarray((s * Wq).T)
    bq_s = np.ascontiguousarray((s * bq).reshape(P, 1))
    wk_t = np.ascontiguousarray(Wk.T)
    bk_c = np.ascontiguousarray(bk.reshape(P, 1))
    wv_t = np.ascontiguousarray(Wv.T)
    bo_f = np.ascontiguousarray((bo + Wo @ bv).reshape(P, 1)).astype(np.float32)
    import ml_dtypes
    d = dict(wq_t=wq_t, bq=bq_s, wk_t=wk_t, bk=bk_c, wv_t=wv_t, bo=bo_f,
             vones=np.ones((P, P), ml_dtypes.bfloat16))
    for h in range(HEADS):
        d[f"wo_{h}"] = np.ascontiguousarray(Wo[:, 32 * h:32 * (h + 1)].T)
    return d


LAST_RESULTS = None


def kernel(x, Wq, bq, Wk, bk, Wv, bv, Wo, bo):
    global LAST_RESULTS
    import os
    from concourse.bass_utils import run_bass_kernel_spmd

    x = np.asarray(x, np.float32)
    B, C, H, W = x.shape
    N = H * W
    key = ("nc", N)
    if key not in _CACHE:
        _CACHE[key] = _build_nc(N)
    nc = _CACHE[key]

    wmap = prep_weights(np.asarray(Wq, np.float32), np.asarray(bq, np.float32),
                        np.asarray(Wk, np.float32), np.asarray(bk, np.float32),
                        np.asarray(Wv, np.float32), np.asarray(bv, np.float32),
                        np.asarray(Wo, np.float32), np.asarray(bo, np.float32))

    in_maps = []
    for b in range(B):
        m = dict(wmap)
        m["x"] = np.ascontiguousarray(x[b].reshape(C, N))
        in_maps.append(m)

    tmpdir = os.environ.get("KERNEL_TMPDIR") or None
    res = run_bass_kernel_spmd(nc, in_maps, core_ids=list(range(B)),
                               tmpdir=tmpdir)
    LAST_RESULTS = res
    out = np.stack([res.results[b]["out"] for b in range(B)], axis=0)
    return out.reshape(B, C, H, W).astype(np.float32)

